# revision 12
# baseline (speedup 1.0000x reference)
"""Trainium2 Bass kernel for nn_Blender (per-style MLP blender).

Strategy
--------
Pure data parallel over the batch: each of the 8 NeuronCores processes
B/8 = 1024 samples with a full replica of the weights. No collectives.

On-chip layout is feature-major ([features -> partitions, batch -> free
dim]) so every GEMM contracts along the partition axis with batch as the
moving dim (N=512 = one fp32 PSUM bank). The host pre-transposes
global_styles to [S, D, B] (fp16) and post-transposes the output back,
so all device DMA is contiguous. The tiny age-MLP (2 MFLOP) is computed
on the host in fp32 and passed in feature-major as fp16.

GEMMs run in fp16 (1 cycle/row, fast weight load) accumulating into
fp32 PSUM; epilogues (bias/relu/residual) run in fp32 on ACT/DVE.

Pipeline per core (BC=1024 samples, chunks of NB=512):
  phase 1: per style group (4 styles column-tiled into the 128-wide PE
           array): bottleneck MLP 512->32->32 with a style-block-
           diagonal second GEMM; accumulate the global MLP's first GEMM
           group by group -> gf2 [128, NB] per chunk.
  phase 2: per style: x = [gs(512) | af(16) | gf2(128)] -> 656->512 GEMM
           + ReLU -> 512->512 GEMM + bias + residual(gs) -> yT.
           gs tiles for the first STASH_S styles stay resident in SBUF
           from phase 1 (no second HBM read).
"""

import numpy as np

import concourse.bacc as bacc
import concourse.tile as tile
from concourse import mybir
from concourse.bass_utils import run_bass_kernel_spmd

S, D, BN, GH, AH, FCH = 18, 512, 32, 128, 16, 512
B = 8192
N_CORES = 8
BC = B // N_CORES          # samples per core
NB = 512                   # moving-dim (batch) tile = one fp32 PSUM bank
N_CHUNKS = BC // NB
GROUPS = [(0, 4), (4, 4), (8, 4), (12, 4), (16, 2)]
KT1 = 6                    # fc1 k-tiles: 4x gs(128) + af(16) + gf2(128)
STASH_S = 14               # styles whose gs tiles stay resident across phases

F32 = mybir.dt.float32
MM_DT = mybir.dt.float16
NP_MM = np.float16

_CACHE = {}


def build_program():
    nc = bacc.Bacc("TRN2", target_bir_lowering=False, debug=False,
                   num_devices=N_CORES)
    mm = nc.tensor.matmul

    din = lambda name, shape, dt=MM_DT: nc.dram_tensor(name, shape, dt, kind="ExternalInput").ap()
    gsT = din("gsT", [S, D, BC])
    afT = din("afT", [AH, BC])
    bn_w1t = din("bn_w1t", [128, S * 4 * BN])
    bn_b1g = din("bn_b1g", [128, len(GROUPS)], F32)
    bn_w2bd = din("bn_w2bd", [128, len(GROUPS) * 128])
    bn_b2g = din("bn_b2g", [128, len(GROUPS)], F32)
    gm_w1g = din("gm_w1g", [128, len(GROUPS) * GH])
    gm_b1 = din("gm_b1", [GH, 1], F32)
    gm_w2 = din("gm_w2", [GH, GH])
    gm_b2 = din("gm_b2", [GH, 1], F32)
    fc_w1t = din("fc_w1t", [S, 128, KT1 * FCH])     # [s, p, kt*512 + h]
    fc_b1t = din("fc_b1t", [S, 128, 4], F32)
    fc_w2t = din("fc_w2t", [S, 128, 16 * 128])      # [s, p, (kt*4+dt)*128 + j]
    fc_b2t = din("fc_b2t", [S, 128, 4], F32)
    yT = nc.dram_tensor("yT", [S, D, BC], F32, kind="ExternalOutput").ap()

    Relu = mybir.ActivationFunctionType.Relu
    Ident = mybir.ActivationFunctionType.Identity
    ADD = mybir.AluOpType.add

    with (
        tile.TileContext(nc) as tc,
        tc.tile_pool(name="consts", bufs=1) as consts,
        tc.tile_pool(name="stash", bufs=1) as stash_pool,
        tc.tile_pool(name="gstr", bufs=2) as gstr_pool,       # streamed gs (styles >= STASH_S)
        tc.tile_pool(name="act1", bufs=3) as act1_pool,
        tc.tile_pool(name="wp", bufs=2) as w_pool,
        tc.tile_pool(name="y1p", bufs=2) as y1_pool,
        tc.tile_pool(name="outp", bufs=4) as out_pool,
        tc.tile_pool(name="ps", bufs=1, space="PSUM") as ps,
    ):
        # ---- resident constants ----
        bn_w1_sb = consts.tile([128, S * 4 * BN], MM_DT, tag="bn_w1")
        nc.sync.dma_start(bn_w1_sb[:], bn_w1t[:])
        bn_b1_sb = consts.tile([128, len(GROUPS)], F32, tag="bn_b1")
        nc.sync.dma_start(bn_b1_sb[:], bn_b1g[:])
        bn_w2_sb = consts.tile([128, len(GROUPS) * 128], MM_DT, tag="bn_w2")
        nc.sync.dma_start(bn_w2_sb[:], bn_w2bd[:])
        bn_b2_sb = consts.tile([128, len(GROUPS)], F32, tag="bn_b2")
        nc.sync.dma_start(bn_b2_sb[:], bn_b2g[:])
        gm_w1_sb = consts.tile([128, len(GROUPS) * GH], MM_DT, tag="gm_w1")
        nc.sync.dma_start(gm_w1_sb[:], gm_w1g[:])
        gm_b1_sb = consts.tile([GH, 1], F32, tag="gm_b1")
        nc.sync.dma_start(gm_b1_sb[:], gm_b1[:])
        gm_w2_sb = consts.tile([GH, GH], MM_DT, tag="gm_w2")
        nc.sync.dma_start(gm_w2_sb[:], gm_w2[:])
        gm_b2_sb = consts.tile([GH, 1], F32, tag="gm_b2")
        nc.sync.dma_start(gm_b2_sb[:], gm_b2[:])
        af_sb = consts.tile([AH, BC], MM_DT, tag="af")
        nc.sync.dma_start(af_sb[:], afT[:])
        gf2_sb = [consts.tile([GH, NB], MM_DT, tag=f"gf2c{c}", name=f"gf2c{c}")
                  for c in range(N_CHUNKS)]

        # ---------------- phase 1: bottleneck + global MLP ----------------
        gs_tiles = {}      # s -> [4 tiles of [128, BC]]
        ps_g1 = [ps.tile([GH, NB], F32, tag=f"g1c{c}", name=f"ps_g1_{c}")
                 for c in range(N_CHUNKS)]
        for gi, (s0, ng) in enumerate(GROUPS):
            pN = 32 * ng
            ps_h1 = [ps.tile([128, NB], F32, tag="h1", bufs=2, name=f"ps_h1_{gi}_{c}")
                     for c in range(N_CHUNKS)]
            for j in range(ng):
                s = s0 + j
                pool = stash_pool if s < STASH_S else gstr_pool
                tiles = []
                for kt in range(4):
                    t = pool.tile([128, BC], MM_DT,
                                  tag=f"gs_{s}_{kt}" if s < STASH_S else f"gsS{kt}",
                                  name=f"gs_{s}_{kt}")
                    nc.sync.dma_start(t[:], gsT[s, kt * 128:(kt + 1) * 128, :])
                    tiles.append(t)
                gs_tiles[s] = tiles
                for c in range(N_CHUNKS):
                    b0 = c * NB
                    for kt in range(4):
                        mm(ps_h1[c][32 * j:32 * j + 32, :],
                           bn_w1_sb[:, (s * 4 + kt) * BN:(s * 4 + kt + 1) * BN],
                           tiles[kt][:, b0:b0 + NB],
                           start=(kt == 0), stop=(kt == 3),
                           tile_position=(0, 32 * j))
            for c in range(N_CHUNKS):
                h1 = act1_pool.tile([128, NB], MM_DT, tag="h1s", name=f"h1_{gi}_{c}")
                nc.scalar.activation(h1[:pN, :], ps_h1[c][:pN, :], Relu,
                                     bias=bn_b1_sb[:pN, gi:gi + 1])
                ps_h2 = ps.tile([128, NB], F32, tag="h2", name=f"ps_h2_{gi}_{c}")
                mm(ps_h2[:pN, :], bn_w2_sb[:pN, gi * 128:gi * 128 + pN], h1[:pN, :])
                gf = act1_pool.tile([128, NB], MM_DT, tag="gfs", name=f"gf_{gi}_{c}")
                nc.scalar.activation(gf[:pN, :], ps_h2[:pN, :], Ident,
                                     bias=bn_b2_sb[:pN, gi:gi + 1])
                mm(ps_g1[c][:], gm_w1_sb[:pN, gi * GH:(gi + 1) * GH], gf[:pN, :],
                   start=(gi == 0), stop=(gi == len(GROUPS) - 1))
        for c in range(N_CHUNKS):
            gmh = act1_pool.tile([GH, NB], MM_DT, tag="gmh", name=f"gmh_{c}")
            nc.scalar.activation(gmh[:], ps_g1[c][:], Relu, bias=gm_b1_sb[:])
            ps_g2 = ps.tile([GH, NB], F32, tag="h2", name=f"ps_g2_{c}")
            mm(ps_g2[:], gm_w2_sb[:], gmh[:])
            nc.scalar.activation(gf2_sb[c][:], ps_g2[:], Ident, bias=gm_b2_sb[:])

        # ---------------- phase 2: per-style fc MLP + residual ----------------
        for s in range(S):
            w1s = w_pool.tile([128, KT1 * FCH], MM_DT, tag="w1", name=f"w1_{s}")
            nc.sync.dma_start(w1s[:], fc_w1t[s, :, :])
            w2s = w_pool.tile([128, 16 * 128], MM_DT, tag="w2", name=f"w2_{s}")
            nc.sync.dma_start(w2s[:], fc_w2t[s, :, :])
            b1s = w_pool.tile([128, 4], F32, tag="b1", name=f"b1_{s}")
            nc.sync.dma_start(b1s[:], fc_b1t[s, :, :])
            b2s = w_pool.tile([128, 4], F32, tag="b2", name=f"b2_{s}")
            nc.sync.dma_start(b2s[:], fc_b2t[s, :, :])

            if s < STASH_S:
                gst = gs_tiles[s]
            else:
                gst = []
                for kt in range(4):
                    g = gstr_pool.tile([128, BC], MM_DT, tag=f"gsS{kt}",
                                       name=f"gs2_{s}_{kt}")
                    nc.sync.dma_start(g[:], gsT[s, kt * 128:(kt + 1) * 128, :])
                    gst.append(g)

            for c in range(N_CHUNKS):
                b0 = c * NB
                y1 = []
                for ht in range(4):
                    h0 = ht * 128
                    ps_y1 = ps.tile([128, NB], F32, tag="y1", name=f"ps_y1_{s}_{c}_{ht}")
                    for kt in range(4):      # gs k-tiles first (no gf2 dep)
                        mm(ps_y1[:],
                           w1s[:, kt * FCH + h0:kt * FCH + h0 + 128],
                           gst[kt][:, b0:b0 + NB],
                           start=(kt == 0), stop=False)
                    mm(ps_y1[:],             # af k-tile (K=16)
                       w1s[:AH, 4 * FCH + h0:4 * FCH + h0 + 128],
                       af_sb[:, b0:b0 + NB],
                       start=False, stop=False)
                    mm(ps_y1[:],             # gf2 k-tile last
                       w1s[:, 5 * FCH + h0:5 * FCH + h0 + 128],
                       gf2_sb[c][:],
                       start=False, stop=True)
                    y1t = y1_pool.tile([128, NB], MM_DT, tag=f"y1_{ht}",
                                       name=f"y1_{s}_{c}_{ht}")
                    nc.scalar.activation(y1t[:], ps_y1[:], Relu, bias=b1s[:, ht:ht + 1])
                    y1.append(y1t)
                for dt_ in range(4):
                    ps_y = ps.tile([128, NB], F32, tag="y", bufs=2,
                                   name=f"ps_y_{s}_{c}_{dt_}")
                    for kt in range(4):
                        mm(ps_y[:],
                           w2s[:, (kt * 4 + dt_) * 128:(kt * 4 + dt_ + 1) * 128],
                           y1[kt][:],
                           start=(kt == 0), stop=(kt == 3))
                    o = out_pool.tile([128, NB], F32, tag="o", name=f"o_{s}_{c}_{dt_}")
                    nc.vector.scalar_tensor_tensor(
                        o[:], ps_y[:], b2s[:, dt_:dt_ + 1],
                        gst[dt_][:, b0:b0 + NB], op0=ADD, op1=ADD)
                    nc.sync.dma_start(yT[s, dt_ * 128:(dt_ + 1) * 128, b0:b0 + NB], o[:])

    nc.compile()
    return nc


def _prep_weights(bn_w1, bn_b1, bn_w2, bn_b2, gm_w1, gm_b1, gm_w2, gm_b2,
                  fc_w1, fc_b1, fc_w2, fc_b2):
    f = np.float32
    h = NP_MM
    nG = len(GROUPS)
    # [p, (s, kt, j)] : bn_w1[s, kt*128+p, j]
    bn_w1t = np.ascontiguousarray(
        bn_w1.reshape(S, 4, 128, BN).transpose(2, 0, 1, 3).reshape(128, S * 4 * BN), h)
    bn_b1g = np.zeros((128, nG), f)
    bn_b2g = np.zeros((128, nG), f)
    bn_w2bd = np.zeros((128, nG * 128), h)
    for gi, (s0, ng) in enumerate(GROUPS):
        for j in range(ng):
            bn_b1g[32 * j:32 * j + 32, gi] = bn_b1[s0 + j]
            bn_b2g[32 * j:32 * j + 32, gi] = bn_b2[s0 + j]
            bn_w2bd[32 * j:32 * j + 32, gi * 128 + 32 * j:gi * 128 + 32 * j + 32] = bn_w2[s0 + j]
    gm_w1p = np.zeros((nG * 128, GH), f)
    gm_w1p[:S * BN] = gm_w1
    gm_w1g = np.ascontiguousarray(
        gm_w1p.reshape(nG, 128, GH).transpose(1, 0, 2).reshape(128, nG * GH), h)
    # fc1 rows reordered to [gs (512) | af (16 at k-tile 4) | gf (128 at k-tile 5)]
    w1p = np.zeros((S, KT1 * 128, FCH), h)
    w1p[:, :4 * 128] = fc_w1[:, GH + AH:]
    w1p[:, 4 * 128:4 * 128 + AH] = fc_w1[:, GH:GH + AH]
    w1p[:, 5 * 128:5 * 128 + GH] = fc_w1[:, :GH]
    fc_w1t = np.ascontiguousarray(
        w1p.reshape(S, KT1, 128, FCH).transpose(0, 2, 1, 3).reshape(S, 128, KT1 * FCH), h)
    fc_b1t = np.ascontiguousarray(fc_b1.reshape(S, 4, 128).transpose(0, 2, 1), f)
    fc_w2t = np.ascontiguousarray(
        fc_w2.reshape(S, 4, 128, 4, 128).transpose(0, 2, 1, 3, 4).reshape(S, 128, 16 * 128), h)
    fc_b2t = np.ascontiguousarray(fc_b2.reshape(S, 4, 128).transpose(0, 2, 1), f)
    return dict(
        bn_w1t=bn_w1t, bn_b1g=bn_b1g, bn_w2bd=bn_w2bd, bn_b2g=bn_b2g,
        gm_w1g=gm_w1g, gm_b1=np.ascontiguousarray(gm_b1.reshape(GH, 1), f),
        gm_w2=np.ascontiguousarray(gm_w2, h),
        gm_b2=np.ascontiguousarray(gm_b2.reshape(GH, 1), f),
        fc_w1t=fc_w1t, fc_b1t=fc_b1t, fc_w2t=fc_w2t, fc_b2t=fc_b2t,
    )


def run(inputs: dict, trace: bool = False):
    """Build in_maps from full inputs, run SPMD on 8 cores, return
    (full_output, BassKernelResults)."""
    if "nc" not in _CACHE:
        _CACHE["nc"] = build_program()
    nc = _CACHE["nc"]

    gs = inputs["global_styles"]
    ages = inputs["target_ages"]
    # host: exact fp32 age MLP (tiny)
    af = np.maximum(ages[:, None] @ inputs["age_w1"] + inputs["age_b1"], 0.0)
    af = af @ inputs["age_w2"] + inputs["age_b2"]          # [B, 16]
    afT_full = np.ascontiguousarray(af.T.astype(NP_MM))
    w = _prep_weights(
        inputs["bn_w1"], inputs["bn_b1"], inputs["bn_w2"], inputs["bn_b2"],
        inputs["gm_w1"], inputs["gm_b1"], inputs["gm_w2"], inputs["gm_b2"],
        inputs["fc_w1"], inputs["fc_b1"], inputs["fc_w2"], inputs["fc_b2"])

    gsT_full = np.ascontiguousarray(gs.transpose(1, 2, 0).astype(NP_MM))  # [S, D, B]
    in_maps = []
    for c in range(N_CORES):
        sl = slice(c * BC, (c + 1) * BC)
        m = dict(w)
        m["gsT"] = np.ascontiguousarray(gsT_full[:, :, sl])
        m["afT"] = np.ascontiguousarray(afT_full[:, sl])
        in_maps.append(m)

    res = run_bass_kernel_spmd(nc, in_maps, core_ids=list(range(N_CORES)),
                               trace=trace)
    yT = np.concatenate([res.results[c]["yT"][:, :, :] for c in range(N_CORES)],
                        axis=2)                              # [S, D, B]
    y = np.ascontiguousarray(yT.transpose(2, 0, 1))          # [B, S, D]
    return y, res


def kernel(**inputs) -> np.ndarray:
    y, _ = run(inputs, trace=False)
    return y


# revision 13
# speedup vs baseline: 1.1713x; 1.1713x over previous
"""Trainium2 Bass kernel for nn_Blender (per-style MLP blender).

Strategy
--------
Pure data parallel over the batch: each of the 8 NeuronCores processes
B/8 = 1024 samples with a full replica of the weights. No collectives.

On-chip layout is feature-major ([features -> partitions, batch -> free
dim]) so every GEMM contracts along the partition axis with batch as the
moving dim (N=512 = one fp32 PSUM bank). The host pre-transposes
global_styles to [S, D, B] (fp16) and post-transposes the output back,
so all device DMA is contiguous. The tiny age-MLP (2 MFLOP) is computed
on the host in fp32 and passed in feature-major as fp16.

GEMMs run in fp16 (1 cycle/row, fast weight load) accumulating into
fp32 PSUM; epilogues (bias/relu/residual) run in fp32 on ACT/DVE.

Pipeline per core (BC=1024 samples, chunks of NB=512):
  phase 1: per style group (4 styles column-tiled into the 128-wide PE
           array): bottleneck MLP 512->32->32 with a style-block-
           diagonal second GEMM; accumulate the global MLP's first GEMM
           group by group -> gf2 [128, NB] per chunk.
  phase 2: per style: x = [gs(512) | af(16) | gf2(128)] -> 656->512 GEMM
           + ReLU -> 512->512 GEMM + bias + residual(gs) -> yT.
           gs tiles for the first STASH_S styles stay resident in SBUF
           from phase 1 (no second HBM read).
"""

import numpy as np

import concourse.bacc as bacc
import concourse.tile as tile
from concourse import mybir
from concourse.bass_utils import run_bass_kernel_spmd

S, D, BN, GH, AH, FCH = 18, 512, 32, 128, 16, 512
B = 8192
N_CORES = 8
BC = B // N_CORES          # samples per core
NB = 512                   # moving-dim (batch) tile = one fp32 PSUM bank
N_CHUNKS = BC // NB
GROUPS = [(0, 4), (4, 4), (8, 4), (12, 4), (16, 2)]
KT1 = 6                    # fc1 k-tiles: 4x gs(128) + af(16) + gf2(128)
STASH_S = 14               # styles whose gs tiles stay resident across phases

F32 = mybir.dt.float32
MM_DT = mybir.dt.float16
NP_MM = np.float16

_CACHE = {}


def build_program():
    nc = bacc.Bacc("TRN2", target_bir_lowering=False, debug=False,
                   num_devices=N_CORES)
    mm = nc.tensor.matmul

    din = lambda name, shape, dt=MM_DT: nc.dram_tensor(name, shape, dt, kind="ExternalInput").ap()
    gsT = din("gsT", [S, D, BC])
    afT = din("afT", [AH, BC])
    bn_w1t = din("bn_w1t", [128, S * 4 * BN])
    bn_b1g = din("bn_b1g", [128, len(GROUPS)], F32)
    bn_w2bd = din("bn_w2bd", [128, len(GROUPS) * 128])
    bn_b2g = din("bn_b2g", [128, len(GROUPS)], F32)
    gm_w1g = din("gm_w1g", [128, len(GROUPS) * GH])
    gm_b1 = din("gm_b1", [GH, 1], F32)
    gm_w2 = din("gm_w2", [GH, GH])
    gm_b2 = din("gm_b2", [GH, 1], F32)
    fc_w1t = din("fc_w1t", [S, 128, KT1 * FCH])     # [s, p, kt*512 + h]
    fc_b1t = din("fc_b1t", [S, 128, 4], F32)
    fc_w2t = din("fc_w2t", [S, 128, 16 * 128])      # [s, p, (kt*4+dt)*128 + j]
    fc_b2t = din("fc_b2t", [S, 128, 4], F32)
    yT = nc.dram_tensor("yT", [S, D, BC], F32, kind="ExternalOutput").ap()

    Relu = mybir.ActivationFunctionType.Relu
    Ident = mybir.ActivationFunctionType.Identity
    ADD = mybir.AluOpType.add

    with (
        tile.TileContext(nc) as tc,
        tc.tile_pool(name="consts", bufs=1) as consts,
        tc.tile_pool(name="stash", bufs=1) as stash_pool,
        tc.tile_pool(name="gstr", bufs=2) as gstr_pool,       # streamed gs (styles >= STASH_S)
        tc.tile_pool(name="act1", bufs=3) as act1_pool,
        tc.tile_pool(name="wp", bufs=2) as w_pool,
        tc.tile_pool(name="y1p", bufs=2) as y1_pool,
        tc.tile_pool(name="outp", bufs=4) as out_pool,
        tc.tile_pool(name="ps", bufs=1, space="PSUM") as ps,
    ):
        # ---- resident constants ----
        bn_w1_sb = consts.tile([128, S * 4 * BN], MM_DT, tag="bn_w1")
        nc.sync.dma_start(bn_w1_sb[:], bn_w1t[:])
        bn_b1_sb = consts.tile([128, len(GROUPS)], F32, tag="bn_b1")
        nc.sync.dma_start(bn_b1_sb[:], bn_b1g[:])
        bn_w2_sb = consts.tile([128, len(GROUPS) * 128], MM_DT, tag="bn_w2")
        nc.sync.dma_start(bn_w2_sb[:], bn_w2bd[:])
        bn_b2_sb = consts.tile([128, len(GROUPS)], F32, tag="bn_b2")
        nc.sync.dma_start(bn_b2_sb[:], bn_b2g[:])
        gm_w1_sb = consts.tile([128, len(GROUPS) * GH], MM_DT, tag="gm_w1")
        nc.sync.dma_start(gm_w1_sb[:], gm_w1g[:])
        gm_b1_sb = consts.tile([GH, 1], F32, tag="gm_b1")
        nc.sync.dma_start(gm_b1_sb[:], gm_b1[:])
        gm_w2_sb = consts.tile([GH, GH], MM_DT, tag="gm_w2")
        nc.sync.dma_start(gm_w2_sb[:], gm_w2[:])
        gm_b2_sb = consts.tile([GH, 1], F32, tag="gm_b2")
        nc.sync.dma_start(gm_b2_sb[:], gm_b2[:])
        af_sb = consts.tile([AH, BC], MM_DT, tag="af")
        nc.sync.dma_start(af_sb[:], afT[:])
        gf2_sb = [consts.tile([GH, NB], MM_DT, tag=f"gf2c{c}", name=f"gf2c{c}")
                  for c in range(N_CHUNKS)]

        # ---------------- phase 1: bottleneck + global MLP ----------------
        gs_tiles = {}      # s -> [4 tiles of [128, BC]]
        ps_g1 = [ps.tile([GH, NB], F32, tag=f"g1c{c}", name=f"ps_g1_{c}")
                 for c in range(N_CHUNKS)]
        for gi, (s0, ng) in enumerate(GROUPS):
            pN = 32 * ng
            ps_h1 = [ps.tile([128, NB], F32, tag="h1", bufs=1, name=f"ps_h1_{gi}_{c}")
                     for c in range(N_CHUNKS)]
            for j in range(ng):
                s = s0 + j
                pool = stash_pool if s < STASH_S else gstr_pool
                tiles = []
                for kt in range(4):
                    t = pool.tile([128, BC], MM_DT,
                                  tag=f"gs_{s}_{kt}" if s < STASH_S else f"gsS{kt}",
                                  name=f"gs_{s}_{kt}")
                    nc.sync.dma_start(t[:], gsT[s, kt * 128:(kt + 1) * 128, :])
                    tiles.append(t)
                gs_tiles[s] = tiles
                for c in range(N_CHUNKS):
                    b0 = c * NB
                    for kt in range(4):
                        mm(ps_h1[c][32 * j:32 * j + 32, :],
                           bn_w1_sb[:, (s * 4 + kt) * BN:(s * 4 + kt + 1) * BN],
                           tiles[kt][:, b0:b0 + NB],
                           start=(kt == 0), stop=(kt == 3),
                           tile_position=(0, 32 * j))
            for c in range(N_CHUNKS):
                h1 = act1_pool.tile([128, NB], MM_DT, tag="h1s", name=f"h1_{gi}_{c}")
                nc.scalar.activation(h1[:pN, :], ps_h1[c][:pN, :], Relu,
                                     bias=bn_b1_sb[:pN, gi:gi + 1])
                ps_h2 = ps.tile([128, NB], F32, tag="h2", name=f"ps_h2_{gi}_{c}")
                mm(ps_h2[:pN, :], bn_w2_sb[:pN, gi * 128:gi * 128 + pN], h1[:pN, :])
                gf = act1_pool.tile([128, NB], MM_DT, tag="gfs", name=f"gf_{gi}_{c}")
                nc.scalar.activation(gf[:pN, :], ps_h2[:pN, :], Ident,
                                     bias=bn_b2_sb[:pN, gi:gi + 1])
                mm(ps_g1[c][:], gm_w1_sb[:pN, gi * GH:(gi + 1) * GH], gf[:pN, :],
                   start=(gi == 0), stop=(gi == len(GROUPS) - 1))
        for c in range(N_CHUNKS):
            gmh = act1_pool.tile([GH, NB], MM_DT, tag="gmh", name=f"gmh_{c}")
            nc.scalar.activation(gmh[:], ps_g1[c][:], Relu, bias=gm_b1_sb[:])
            ps_g2 = ps.tile([GH, NB], F32, tag="h2", name=f"ps_g2_{c}")
            mm(ps_g2[:], gm_w2_sb[:], gmh[:])
            nc.scalar.activation(gf2_sb[c][:], ps_g2[:], Ident, bias=gm_b2_sb[:])

        # ---------------- phase 2: per-style fc MLP + residual ----------------
        for s in range(S):
            w1s = w_pool.tile([128, KT1 * FCH], MM_DT, tag="w1", name=f"w1_{s}")
            nc.sync.dma_start(w1s[:], fc_w1t[s, :, :])
            w2s = w_pool.tile([128, 16 * 128], MM_DT, tag="w2", name=f"w2_{s}")
            nc.sync.dma_start(w2s[:], fc_w2t[s, :, :])
            b1s = w_pool.tile([128, 4], F32, tag="b1", name=f"b1_{s}")
            nc.sync.dma_start(b1s[:], fc_b1t[s, :, :])
            b2s = w_pool.tile([128, 4], F32, tag="b2", name=f"b2_{s}")
            nc.sync.dma_start(b2s[:], fc_b2t[s, :, :])

            if s < STASH_S:
                gst = gs_tiles[s]
            else:
                gst = []
                for kt in range(4):
                    g = gstr_pool.tile([128, BC], MM_DT, tag=f"gsS{kt}",
                                       name=f"gs2_{s}_{kt}")
                    nc.sync.dma_start(g[:], gsT[s, kt * 128:(kt + 1) * 128, :])
                    gst.append(g)

            for c in range(N_CHUNKS):
                b0 = c * NB
                y1 = []
                for ht in range(4):
                    h0 = ht * 128
                    ps_y1 = ps.tile([128, NB], F32, tag="y1", bufs=2, name=f"ps_y1_{s}_{c}_{ht}")
                    for kt in range(4):      # gs k-tiles first (no gf2 dep)
                        mm(ps_y1[:],
                           w1s[:, kt * FCH + h0:kt * FCH + h0 + 128],
                           gst[kt][:, b0:b0 + NB],
                           start=(kt == 0), stop=False)
                    mm(ps_y1[:],             # af k-tile (K=16)
                       w1s[:AH, 4 * FCH + h0:4 * FCH + h0 + 128],
                       af_sb[:, b0:b0 + NB],
                       start=False, stop=False)
                    mm(ps_y1[:],             # gf2 k-tile last
                       w1s[:, 5 * FCH + h0:5 * FCH + h0 + 128],
                       gf2_sb[c][:],
                       start=False, stop=True)
                    y1t = y1_pool.tile([128, NB], MM_DT, tag=f"y1_{ht}",
                                       name=f"y1_{s}_{c}_{ht}")
                    nc.scalar.activation(y1t[:], ps_y1[:], Relu, bias=b1s[:, ht:ht + 1])
                    y1.append(y1t)
                for dt_ in range(4):
                    ps_y = ps.tile([128, NB], F32, tag="y", bufs=2,
                                   name=f"ps_y_{s}_{c}_{dt_}")
                    for kt in range(4):
                        mm(ps_y[:],
                           w2s[:, (kt * 4 + dt_) * 128:(kt * 4 + dt_ + 1) * 128],
                           y1[kt][:],
                           start=(kt == 0), stop=(kt == 3))
                    o = out_pool.tile([128, NB], F32, tag="o", name=f"o_{s}_{c}_{dt_}")
                    nc.vector.scalar_tensor_tensor(
                        o[:], ps_y[:], b2s[:, dt_:dt_ + 1],
                        gst[dt_][:, b0:b0 + NB], op0=ADD, op1=ADD)
                    nc.sync.dma_start(yT[s, dt_ * 128:(dt_ + 1) * 128, b0:b0 + NB], o[:])

    nc.compile()
    return nc


def _prep_weights(bn_w1, bn_b1, bn_w2, bn_b2, gm_w1, gm_b1, gm_w2, gm_b2,
                  fc_w1, fc_b1, fc_w2, fc_b2):
    f = np.float32
    h = NP_MM
    nG = len(GROUPS)
    # [p, (s, kt, j)] : bn_w1[s, kt*128+p, j]
    bn_w1t = np.ascontiguousarray(
        bn_w1.reshape(S, 4, 128, BN).transpose(2, 0, 1, 3).reshape(128, S * 4 * BN), h)
    bn_b1g = np.zeros((128, nG), f)
    bn_b2g = np.zeros((128, nG), f)
    bn_w2bd = np.zeros((128, nG * 128), h)
    for gi, (s0, ng) in enumerate(GROUPS):
        for j in range(ng):
            bn_b1g[32 * j:32 * j + 32, gi] = bn_b1[s0 + j]
            bn_b2g[32 * j:32 * j + 32, gi] = bn_b2[s0 + j]
            bn_w2bd[32 * j:32 * j + 32, gi * 128 + 32 * j:gi * 128 + 32 * j + 32] = bn_w2[s0 + j]
    gm_w1p = np.zeros((nG * 128, GH), f)
    gm_w1p[:S * BN] = gm_w1
    gm_w1g = np.ascontiguousarray(
        gm_w1p.reshape(nG, 128, GH).transpose(1, 0, 2).reshape(128, nG * GH), h)
    # fc1 rows reordered to [gs (512) | af (16 at k-tile 4) | gf (128 at k-tile 5)]
    w1p = np.zeros((S, KT1 * 128, FCH), h)
    w1p[:, :4 * 128] = fc_w1[:, GH + AH:]
    w1p[:, 4 * 128:4 * 128 + AH] = fc_w1[:, GH:GH + AH]
    w1p[:, 5 * 128:5 * 128 + GH] = fc_w1[:, :GH]
    fc_w1t = np.ascontiguousarray(
        w1p.reshape(S, KT1, 128, FCH).transpose(0, 2, 1, 3).reshape(S, 128, KT1 * FCH), h)
    fc_b1t = np.ascontiguousarray(fc_b1.reshape(S, 4, 128).transpose(0, 2, 1), f)
    fc_w2t = np.ascontiguousarray(
        fc_w2.reshape(S, 4, 128, 4, 128).transpose(0, 2, 1, 3, 4).reshape(S, 128, 16 * 128), h)
    fc_b2t = np.ascontiguousarray(fc_b2.reshape(S, 4, 128).transpose(0, 2, 1), f)
    return dict(
        bn_w1t=bn_w1t, bn_b1g=bn_b1g, bn_w2bd=bn_w2bd, bn_b2g=bn_b2g,
        gm_w1g=gm_w1g, gm_b1=np.ascontiguousarray(gm_b1.reshape(GH, 1), f),
        gm_w2=np.ascontiguousarray(gm_w2, h),
        gm_b2=np.ascontiguousarray(gm_b2.reshape(GH, 1), f),
        fc_w1t=fc_w1t, fc_b1t=fc_b1t, fc_w2t=fc_w2t, fc_b2t=fc_b2t,
    )


def run(inputs: dict, trace: bool = False):
    """Build in_maps from full inputs, run SPMD on 8 cores, return
    (full_output, BassKernelResults)."""
    if "nc" not in _CACHE:
        _CACHE["nc"] = build_program()
    nc = _CACHE["nc"]

    gs = inputs["global_styles"]
    ages = inputs["target_ages"]
    # host: exact fp32 age MLP (tiny)
    af = np.maximum(ages[:, None] @ inputs["age_w1"] + inputs["age_b1"], 0.0)
    af = af @ inputs["age_w2"] + inputs["age_b2"]          # [B, 16]
    afT_full = np.ascontiguousarray(af.T.astype(NP_MM))
    w = _prep_weights(
        inputs["bn_w1"], inputs["bn_b1"], inputs["bn_w2"], inputs["bn_b2"],
        inputs["gm_w1"], inputs["gm_b1"], inputs["gm_w2"], inputs["gm_b2"],
        inputs["fc_w1"], inputs["fc_b1"], inputs["fc_w2"], inputs["fc_b2"])

    gsT_full = np.ascontiguousarray(gs.transpose(1, 2, 0).astype(NP_MM))  # [S, D, B]
    in_maps = []
    for c in range(N_CORES):
        sl = slice(c * BC, (c + 1) * BC)
        m = dict(w)
        m["gsT"] = np.ascontiguousarray(gsT_full[:, :, sl])
        m["afT"] = np.ascontiguousarray(afT_full[:, sl])
        in_maps.append(m)

    res = run_bass_kernel_spmd(nc, in_maps, core_ids=list(range(N_CORES)),
                               trace=trace)
    yT = np.concatenate([res.results[c]["yT"][:, :, :] for c in range(N_CORES)],
                        axis=2)                              # [S, D, B]
    y = np.ascontiguousarray(yT.transpose(2, 0, 1))          # [B, S, D]
    return y, res


def kernel(**inputs) -> np.ndarray:
    y, _ = run(inputs, trace=False)
    return y
